# revision 17
# baseline (speedup 1.0000x reference)
"""AlignBlock kernel for 8 TRN2 NeuronCores.

Reference computation (B=2, C=2, T=500, F=129, H=16, D=100):
  Q = conv1x1(x_mic; w_mic, b_mic)        (B,H,T,F)
  K = conv1x1(x_ref; w_ref, b_ref)        (B,H,T,F)
  V[b,h,t,d]  = sum_f Q[b,h,t,f] * Kpad[b,h,t-99+d,f]       (delay window)
  V2 = conv2d(V, w_conv (1,H,5,3), causal-T pad (4,0), d pad (1,1)) + b_conv
  A  = softmax_d(V2[:,0])                 (B,T,D)
  y[b,c,t,f] = sum_d x_refpad[b,c,t-99+d,f] * A[b,t,d]

Key algebraic restructuring (all exact):
  - The H dimension is folded on the host: with augmented channels
    xm~ = [xm0, xm1, u], xr~ = [xr0, xr1, v] (u/v = validity masks emulating
    the reference's zero padding of Q rows / K columns), the conv input
    planes are sum_h w_conv[h]*V[h] = sum_{q=(cm,cr)} Wc[q] * XC[q] where
    XC[q][t,t'] = sum_f xm~[cm,t,f] xr~[cr,t',f]  (9 raw correlation planes)
    and Wc[q,i,j] = sum_h w_conv[h,i,j] wm~[h,cm] wr~[h,cr].
  - The causal 5-tap T conv becomes banded-matrix matmuls (contraction over
    conv input rows); the 3 d-taps are free-dim shifted column reads.
  - softmax(V2 + b_conv) == softmax(V2): b_conv is dropped.
  - y is a matmul contracting t' with the banded attention matrix A_band.

All data movement is kept on-chip (no DRAM scratch round-trips):
  - The diagonal regather XC[r, r+d] is a single SBUF->SBUF DMA per
    correlation group with a hand-built diagonal access pattern
    (stride = partition_row + 1 element).
  - The attention banding A_band[tau, tau+4+d] = A[tau, d] is an
    SBUF->SBUF DMA with a skewed destination AP, and the transpose to
    [t', tau] runs on the tensor engine (identity matmul), not the DMA
    xbar, so nothing touches HBM between the input load and output store.

Sharding: sequence-parallel over T, 63 output frames/core (T padded 500->504),
each core loads its input slice with halos host-side; no collectives.
"""

import os
import sys

import numpy as np

sys.path.insert(0, "/opt/trn_rl_repo")

# ---- problem constants (hardcoded per the staged problem) ----
B, C, T, F = 2, 2, 500, 129
H, D = 16, 100
NCORES = 8
TL = 63               # output frames per core
TP = NCORES * TL      # padded T = 504
R = TL + 4            # conv input rows per core (67)
TH = TL + D + 3       # x_ref halo columns per core (166)
NQ = 9                # augmented channel pairs
DW = D + 2            # padded delay width incl. zero edge cols (102)
NG = 3                # q-triplet groups
GQ = NQ // NG

_CACHE = {}


def _np_reference(x_mic, x_ref, w_mic, b_mic, w_ref, b_ref, w_conv, b_conv, delay):
    """Pure-numpy fallback, exact mirror of the jax reference."""
    Bn, Cn, Tn, Fn = x_mic.shape
    Dn = int(delay)
    Q = np.einsum("bctf,hc->bhtf", x_mic, w_mic) + b_mic[None, :, None, None]
    K = np.einsum("bctf,hc->bhtf", x_ref, w_ref) + b_ref[None, :, None, None]
    idx = np.arange(Tn)[:, None] + np.arange(Dn)[None, :]
    Kp = np.pad(K, ((0, 0), (0, 0), (Dn - 1, 0), (0, 0)))
    Ku = Kp[:, :, idx, :]
    V = np.einsum("bhtf,bhtdf->bhtd", Q, Ku)
    Vp = np.pad(V, ((0, 0), (0, 0), (4, 0), (1, 1)))
    out = np.zeros((Bn, Tn, Dn), np.float32)
    for i in range(5):
        for j in range(3):
            out += np.einsum(
                "bhtd,h->btd", Vp[:, :, i : i + Tn, j : j + Dn], w_conv[0, :, i, j]
            )
    out += b_conv[0]
    m = out.max(-1, keepdims=True)
    e = np.exp(out - m)
    A = e / e.sum(-1, keepdims=True)
    Rp = np.pad(x_ref, ((0, 0), (0, 0), (Dn - 1, 0), (0, 0)))
    Ru = Rp[:, :, idx, :]
    return np.einsum("bctdf,btd->bctf", Ru, A).astype(np.float32)


def _diag_ap(tile_ap, dims, offset):
    """Hand-built access pattern on an SBUF tile (flat element strides;
    strides may mix partition steps with in-partition offsets)."""
    a = tile_ap.copy()
    VecI64Pair = type(a.ap)
    a.ap = VecI64Pair([list(d) for d in dims])
    a.offset = offset
    return a


def _build_graph():
    """Build + compile the single-core SPMD Bass graph (identical on all cores)."""
    from concourse import bacc, mybir, tile

    dt = mybir.dt
    f32 = dt.float32
    bf16 = dt.bfloat16

    nc = bacc.Bacc(
        "TRN2", target_bir_lowering=False, debug=False, num_devices=NCORES
    )

    # external I/O (per-core shards, host-prepared layouts).
    # Every large tensor is shaped with 128 partition rows: the HWDGE only
    # spreads a transfer's descriptors across the 16 SDMA engines when the
    # partition dim is 128 (67-row loads serialize on one engine).
    xcmb = nc.dram_tensor("xcmb", [128, B, 3, R + TH], bf16, kind="ExternalInput")
    xtra = nc.dram_tensor("xtra", [1, B, 3, R + TH], bf16, kind="ExternalInput")
    xrnp = nc.dram_tensor("xrnp", [128, 2, B, C, F], bf16, kind="ExternalInput")
    bcv = nc.dram_tensor("bcv", [128, NQ, 3, TL], bf16, kind="ExternalInput")
    outs = [
        nc.dram_tensor(f"out{b}", [TL, C, F], bf16, kind="ExternalOutput")
        for b in range(B)
    ]

    with tile.TileContext(nc) as tc:
        with (
            tc.tile_pool(name="w", bufs=1) as wp,
            tc.tile_pool(name="xcps", bufs=3, space="PSUM") as xcp,
            tc.tile_pool(name="cvps", bufs=1, space="PSUM") as cvp,
            tc.tile_pool(name="tpps", bufs=2, space="PSUM") as tpp,
            tc.tile_pool(name="yps", bufs=2, space="PSUM") as yp,
            tc.tile_pool(name="st", bufs=6) as sp,
            tc.tile_pool(name="sm", bufs=2) as smp,
        ):
            # ---- persistent input tiles ----
            xcb = wp.tile([128, B, 3, R + TH], bf16, tag="xcb")
            xtr = wp.tile([1, B, 3, R + TH], bf16, tag="xtr")
            xrnt = wp.tile([128, 2, B, C, F], bf16, tag="xrnt")
            bcw = wp.tile([128, NQ, 3, TL], bf16, tag="bcw")

            nc.sync.dma_start(out=xcb[:], in_=xcmb.ap())
            nc.sync.dma_start(out=xtr[:], in_=xtra.ap())
            nc.scalar.dma_start(out=bcw[:], in_=bcv.ap())

            # on-chip scratch (gpsimd, no input deps). xcd edge columns
            # (dw = 0 and DW-1) must stay zero; the band is DMA-filled.
            xcd = wp.tile([R, B, NG, DW, GQ], bf16, tag="xcd")
            nc.gpsimd.memset(xcd[:], 0.0)
            sgb = [
                wp.tile([64, 256], bf16, tag=f"sgb{b}", name=f"sgb{b}")
                for b in range(B)
            ]
            for b in range(B):
                nc.gpsimd.memset(sgb[b][:], 0.0)
            ident = wp.tile([64, 64], bf16, tag="ident")
            nc.gpsimd.memset(ident[:], 1.0)
            nc.gpsimd.affine_select(
                out=ident[:], in_=ident[:], pattern=[[-1, 64]], base=0,
                channel_multiplier=1, compare_op=mybir.AluOpType.is_equal,
                fill=0.0,
            )

            nc.scalar.dma_start(out=xrnt[:], in_=xrnp.ap())

            xsbs = [
                sp.tile([R, TH, GQ], bf16, tag=f"xsb{i}", name=f"xsb{i}")
                for i in range(6)
            ]

            # PE warm-up: dense dummy matmuls during the DMA load prologue to
            # release the HAM clock gate before real matmuls.
            wsrc = wp.tile([128, GQ * TH], bf16, tag="wsrc")
            nc.vector.memset(wsrc[:], 0.0)
            for _ in range(8):
                wps = xcp.tile([R, GQ, TH], f32, tag="pxc")
                nc.tensor.matmul(
                    out=wps[:], lhsT=wsrc[:, 0:R], rhs=wsrc[:],
                    start=True, stop=True,
                )

            # ---- stage 1: correlation planes + on-chip diagonal regather ----
            # order (g-major across b) so stage-2 group g unblocks earliest
            ks = [(0, 0), (1, 0), (0, 1), (1, 1), (0, 2), (1, 2)]
            # each PSUM->SBUF cast is split into two column halves so DVE
            # and ACT each move half; the diagonal regathers ride the HWDGE
            # (sync/scalar) whose small-row SBUF->SBUF completions are fast.
            # (The HWDGE 128-row spray path and all SWDGE SBUF->SBUF paths
            # are either incorrect or several us slower here.)
            TH2 = TH // 2
            diag_eng = [nc.gpsimd] * 6
            for i, (b, g) in enumerate(ks):
                pxc = xcp.tile([R, GQ, TH], f32, tag="pxc")
                nc.tensor.matmul(
                    out=pxc[:], lhsT=xcb[:, b, g, 0:R],
                    rhs=xcb[:, b, :, R : R + TH], start=True, stop=False,
                )
                nc.tensor.matmul(
                    out=pxc[:], lhsT=xtr[0:1, b, g, 0:R],
                    rhs=xtr[0:1, b, :, R : R + TH], start=False, stop=True,
                )
                xsb = xsbs[i]
                nc.vector.tensor_copy(
                    out=xsb[:, 0:TH2, :].transpose([0, 2, 1]),
                    in_=pxc[:, :, 0:TH2],
                )
                nc.scalar.copy(
                    xsb[:, TH2:TH, :].transpose([0, 2, 1]),
                    pxc[:, :, TH2:TH],
                )
                # regather: xcd[r, b, g, 1+d, qs] = XC[r, r+d, qs]
                diag_eng[i].dma_start(
                    out=xcd[:, b, g, 1 : 1 + D, :],
                    in_=_diag_ap(
                        xsb[:], [[(TH + 1) * GQ, R], [1, D * GQ]], 0
                    ),
                )

            # ---- stage 2: folded conv as banded matmuls, grouped ----
            v2 = cvp.tile([TL, B, D], f32, tag="v2")
            n_mm = NQ * 3
            k = 0
            for g in range(NG):
                for qs in range(GQ):
                    q = g * GQ + qs
                    for j in range(3):
                        nc.tensor.matmul(
                            out=v2[:],
                            lhsT=bcw[0:R, q, j, :],
                            rhs=xcd[:, :, g, j : j + D, qs],
                            start=(k == 0), stop=(k == n_mm - 1),
                        )
                        k += 1

            # ---- stage 3: softmax over delay, per batch ----
            # (no max subtraction: |logits| stay well under exp's f32 range)
            atts = []
            sums = []
            for b in range(B):
                ex = smp.tile([TL, D], f32, tag="ex")
                ssum = smp.tile([TL, 1], f32, tag="ssum")
                nc.scalar.activation(
                    out=ex[:], in_=v2[:, b, :],
                    func=mybir.ActivationFunctionType.Exp,
                    bias=0.0, scale=1.0, accum_out=ssum[:],
                )
                sums.append((ex, ssum))
            for b in range(B):
                ex, ssum = sums[b]
                rin = smp.tile([TL, 1], f32, tag="rin")
                nc.vector.reciprocal(rin[:], ssum[:])
                att = smp.tile([TL, D], bf16, tag="att")
                nc.vector.tensor_scalar_mul(att[:], ex[:], rin[:])
                atts.append(att)
                # banding: sgb[tau, tau+4+d] = A[tau, d] (skewed SBUF write)
                nc.sync.dma_start(
                    out=_diag_ap(sgb[b][:], [[257, TL], [1, D]], 4),
                    in_=att[:],
                )

            # ---- stage 4: transpose band on the PE, then banded matmul ----
            yout = [
                wp.tile([TL, C, F], bf16, tag=f"yout{b}", name=f"yout{b}")
                for b in range(B)
            ]
            out_eng = [nc.sync, nc.sync]
            for b in range(B):
                a0 = wp.tile([128, 64], bf16, tag=f"a0_{b}")
                a1 = wp.tile([128, 64], bf16, tag=f"a1_{b}")
                for half, atile in enumerate((a0, a1)):
                    aT = tpp.tile([128, 64], bf16, tag="aT")
                    nc.tensor.transpose(
                        aT[:], sgb[b][:, 128 * half : 128 * (half + 1)],
                        ident[:],
                    )
                    nc.vector.tensor_copy(out=atile[:], in_=aT[:])
                py = yp.tile([TL, C, F], f32, tag="py")
                nc.tensor.matmul(
                    out=py[:], lhsT=a0[:, 0:TL], rhs=xrnt[:, 0, b, :, :],
                    start=True, stop=False,
                )
                nc.tensor.matmul(
                    out=py[:], lhsT=a1[0 : TH - 128, 0:TL],
                    rhs=xrnt[0 : TH - 128, 1, b, :, :],
                    start=False, stop=True,
                )
                nc.vector.tensor_copy(out=yout[b][:], in_=py[:])
                out_eng[b].dma_start(out=outs[b].ap(), in_=yout[b][:])

    nc.compile()
    return nc


def _prepare_inputs(x_mic, x_ref, w_mic, b_mic, w_ref, b_ref, w_conv):
    """Host-side sharding + weight folding. Returns in_maps (one dict/core)."""
    from ml_dtypes import bfloat16

    # padded arrays: xm rows [t0-4, t0+63), xr cols [t0-103, t0+63)
    xm_pad = np.zeros((B, C, 4 + TP, F), np.float32)
    xm_pad[:, :, 4 : 4 + T] = x_mic
    xr_pad = np.zeros((B, C, D + 3 + TP, F), np.float32)
    xr_pad[:, :, D + 3 : D + 3 + T] = x_ref

    # folded conv weights: Wc[cm, cr, i, j] = sum_h w_conv * wm~ * wr~
    wt = np.asarray(w_conv, np.float64)[0]          # (H, 5, 3)
    wtm = np.concatenate([w_mic, b_mic[:, None]], 1).astype(np.float64)  # (H,3)
    wtr = np.concatenate([w_ref, b_ref[:, None]], 1).astype(np.float64)  # (H,3)
    Wc = np.einsum("hij,hm,hr->mrij", wt, wtm, wtr)  # (3,3,5,3)

    # banded conv matrices bcv[r, q, j, tau] = Wc[q, r-tau, j]
    # (zero-padded to 128 rows so the load spreads over all SDMA engines)
    bcv = np.zeros((128, 3, 3, 3, TL), np.float32)
    for i in range(5):
        for j in range(3):
            bcv[np.arange(TL) + i, :, :, j, np.arange(TL)] = np.float32(
                Wc[:, :, i, j]
            )[None]
    bcv = bcv.reshape(128, NQ, 3, TL).astype(bfloat16)

    in_maps = []
    for i in range(NCORES):
        t0 = i * TL
        xm_s = xm_pad[:, :, t0 : t0 + R]          # (B,C,R,F) rows t0-4..t0+62
        xr_s = xr_pad[:, :, t0 : t0 + TH]         # (B,C,TH,F) cols t0-103..t0+62
        u = (np.arange(R) + t0 - 4 >= 0).astype(np.float32)
        v = (np.arange(TH) + t0 - D - 3 >= 0).astype(np.float32)

        xmt = np.empty((B, 3, R, F), np.float32)
        xmt[:, :C] = xm_s
        xmt[:, C] = u[:, None]
        xmt = np.ascontiguousarray(xmt.transpose(3, 0, 1, 2)).astype(bfloat16)

        xrt = np.empty((B, 3, TH, F), np.float32)
        xrt[:, :C] = xr_s
        xrt[:, C] = v[:, None]
        xrt = np.ascontiguousarray(xrt.transpose(3, 0, 1, 2)).astype(bfloat16)

        # mic rows and ref cols combined on the last axis, [128, B, 3, R+TH];
        # F row 128 of both goes into the single-partition xtra tensor
        xcmb = np.concatenate([xmt[0:128], xrt[0:128]], axis=-1)
        xtra = np.concatenate([xmt[128], xrt[128]], axis=-1)[None]

        # x_ref halo rows folded to 128 partitions: [p, 0] = row p,
        # [p, 1] = row 128+p (zero beyond TH)
        xrn = xr_s.transpose(2, 0, 1, 3).astype(bfloat16)  # (TH, B, C, F)
        xrnp = np.zeros((128, 2, B, C, F), bfloat16)
        xrnp[:, 0] = xrn[0:128]
        xrnp[: TH - 128, 1] = xrn[128:TH]

        in_maps.append(
            {
                "xcmb": np.ascontiguousarray(xcmb),
                "xtra": np.ascontiguousarray(xtra),
                "xrnp": xrnp,
                "bcv": bcv,
            }
        )
    return in_maps


def kernel(**inputs):
    x_mic = np.asarray(inputs["x_mic"], np.float32)
    x_ref = np.asarray(inputs["x_ref"], np.float32)
    w_mic = np.asarray(inputs["w_mic"], np.float32)
    b_mic = np.asarray(inputs["b_mic"], np.float32)
    w_ref = np.asarray(inputs["w_ref"], np.float32)
    b_ref = np.asarray(inputs["b_ref"], np.float32)
    w_conv = np.asarray(inputs["w_conv"], np.float32)
    b_conv = np.asarray(inputs["b_conv"], np.float32)
    delay = int(np.asarray(inputs["delay"]))

    if (
        x_mic.shape != (B, C, T, F)
        or x_ref.shape != (B, C, T, F)
        or delay != D
        or w_conv.shape != (1, H, 5, 3)
    ):
        return _np_reference(
            x_mic, x_ref, w_mic, b_mic, w_ref, b_ref, w_conv, b_conv, delay
        )

    from concourse.bass_utils import run_bass_kernel_spmd

    if "nc" not in _CACHE:
        _CACHE["nc"] = _build_graph()
    nc = _CACHE["nc"]

    in_maps = _prepare_inputs(x_mic, x_ref, w_mic, b_mic, w_ref, b_ref, w_conv)
    res = run_bass_kernel_spmd(nc, in_maps, core_ids=list(range(NCORES)))

    y = np.zeros((B, C, TP, F), np.float32)
    for i in range(NCORES):
        for b in range(B):
            y[b, :, i * TL : (i + 1) * TL] = res.results[i][f"out{b}"].transpose(
                1, 0, 2
            )
    return np.ascontiguousarray(y[:, :, :T]).astype(np.float32)


if __name__ == "__main__":
    rng = np.random.default_rng(0)
    ins = {
        "x_mic": rng.standard_normal((B, C, T, F), np.float32),
        "x_ref": rng.standard_normal((B, C, T, F), np.float32),
        "w_mic": rng.standard_normal((H, C), np.float32) * 0.5,
        "b_mic": rng.standard_normal((H,), np.float32) * 0.1,
        "w_ref": rng.standard_normal((H, C), np.float32) * 0.5,
        "b_ref": rng.standard_normal((H,), np.float32) * 0.1,
        "w_conv": rng.standard_normal((1, H, 5, 3), np.float32) * 0.05,
        "b_conv": rng.standard_normal((1,), np.float32) * 0.1,
        "delay": D,
    }
    got = kernel(**ins)
    want = _np_reference(**ins)
    err = np.linalg.norm(got - want) / np.linalg.norm(want)
    print("rel err vs numpy ref:", err)
